# revision 18
# baseline (speedup 1.0000x reference)
"""Trainium2 Bass kernel: batched scaled-dot-product attention.

reference: out[b] = softmax(scale * x1[b] @ x2[b].T) @ x3[b]
shapes: x1,x2,x3 = [16, 2048, 128] fp32.

Sharding: B=16 batches data-parallel over 8 NeuronCores (2 batches/core).

Device algorithm (per batch, per q-half of 1024), software-pipelined over a
single global chunk stream so the PE never parks behind a PV matmul that is
waiting on its exp:
  chunk k:  S^T[k,q] = matmul(lhsT=K^T chunk fp16, rhs=Q^T half fp16)  (FWL)
            es(k)    = exp(scale * S^T)  PSUM -> SBUF bf16   ScalarE
            PV(k-1): outT[dv,q] += matmul(lhsT=V chunk bf16, rhs=es(k-1))
  softmax denominators: bf16 chain adds on DVE (2x perf mode) into two
  accumulators per half + last two chunks raw; ones-matmul reduces over
  partitions and broadcasts; rcp on DVE; normalize + store per 512 cols.

Precision: QK in fp16 (10-bit mantissa; scores |s|<~35 so no overflow),
V/es in bf16. Measured end-to-end rel err ~3e-3 vs the 2e-2 gate.

Host side does layout only: pre-transpose Q,K -> fp16; interleave V rows ->
bf16; transpose the output back. All FLOPs run on device.
"""
import os
import sys
import types
import numpy as np
from contextlib import ExitStack

import concourse.bass as bass  # noqa: F401
import concourse.bass_isa as bass_isa
from concourse import bacc
import concourse.mybir as mybir
import concourse.tile as tile
from concourse.tile_rust import add_dep_helper
import concourse.bass_utils as bass_utils
from concourse.bass_utils import run_bass_kernel_spmd

f32 = mybir.dt.float32
f32r = mybir.dt.float32r
f16 = mybir.dt.float16
bf16 = mybir.dt.bfloat16

B, SQ, SK, D = 16, 2048, 2048, 128
NCORES = 8
BPC = B // NCORES  # batches per core
KC = SK // 128     # k chunks
NH = 2             # q halves
HW_ = SQ // NH     # 1024
NRAW = 1           # last NRAW chunks fed raw to the sums matmul
ACC_SPLIT = 8      # chunks 1..ACC_SPLIT-1 -> acc_a, ACC_SPLIT..KC-NRAW-1 -> acc_b


def _patch_ldw_opt():
    """Enable walrus LDWEIGHTS optimization (background weight-buffer overlap)."""
    if getattr(bass_utils, "_ldw_patched", False):
        return
    # fp16/bf16 weights get FWL (background weight load) from the compiler;
    # the walrus ldw-opt flag is both unnecessary and incompatible with it.
    if not bool(int(os.environ.get("KERNEL_LDWOPT", "0"))):
        return
    orig = bass_utils.run_command

    def patched(argv, **kw):
        if isinstance(argv, list):
            argv = [
                "--enable-ldw-opt=true" if a == "--enable-ldw-opt=false" else a
                for a in argv
            ]
        return orig(argv, **kw)

    bass_utils.run_command = patched
    bass_utils._ldw_patched = True


def _install_ntff_hook():
    """Register the axon NTFF profile hook (used only when tracing)."""
    try:
        from antenv import axon_hooks  # noqa: F401
        return
    except ImportError:
        pass
    try:
        m = types.ModuleType("antenv.axon_hooks")
        m._hook = None
        m.set_axon_ntff_profile_hook = lambda h: setattr(m, "_hook", h)
        m.get_axon_ntff_profile_hook = lambda: m._hook
        sys.modules["antenv.axon_hooks"] = m
        import antenv
        antenv.axon_hooks = m
        from trn_agent_boot.trn_boot import _ntff_profile_via_ctypes
        m._hook = _ntff_profile_via_ctypes("/opt/axon/libaxon_pjrt.so")
    except Exception:
        pass


def build(scale: float):
    _patch_ldw_opt()
    nc = bacc.Bacc("TRN2", target_bir_lowering=False, debug=False)
    qt = nc.dram_tensor("qt", [BPC, 128, SQ], f16, kind="ExternalInput")
    kt = nc.dram_tensor("kt", [BPC, 128, SK], f16, kind="ExternalInput")
    vv = nc.dram_tensor("v", [BPC, 128, SK], bf16, kind="ExternalInput")
    ones = nc.dram_tensor("ones", [128, 128], bf16, kind="ExternalInput")
    oo = nc.dram_tensor("o", [BPC, 128, SQ], f32, kind="ExternalOutput")

    Exp = mybir.ActivationFunctionType.Exp

    with tile.TileContext(nc) as tc, ExitStack() as ctx:
        inp = ctx.enter_context(tc.tile_pool(name="inp", bufs=2))
        es_pool = ctx.enter_context(tc.tile_pool(name="es", bufs=12))
        acc_pool = ctx.enter_context(tc.tile_pool(name="acc", bufs=2))
        out_pool = ctx.enter_context(tc.tile_pool(name="out", bufs=2))
        cpool = ctx.enter_context(tc.tile_pool(name="const", bufs=1))
        psS = ctx.enter_context(tc.tile_pool(name="psS", bufs=2, space="PSUM"))
        psO = ctx.enter_context(tc.tile_pool(name="psO", bufs=2, space="PSUM"))

        ones_sb = cpool.tile([128, 128], bf16, tag="ones")
        tail_prio = int(os.environ.get("KERNEL_TAILPRIO", "40"))

        # ---- input DMA: spread issue across Sync/Vector/GpSimd queues so the
        # first chunk's operands land ~2us in instead of serializing on Sync.
        qt_sb, kt_sb, v_sb = [None] * BPC, [None] * BPC, [None] * BPC
        for b in range(BPC):
            qt_sb[b] = inp.tile([128, SQ], f16, tag="qt", name=f"qt_sb{b}")
            kt_sb[b] = inp.tile([128, SK], f16, tag="kt", name=f"kt_sb{b}")
            v_sb[b] = inp.tile([128, SK], bf16, tag="v", name=f"v_sb{b}")
        qa, ka, va = qt.ap()[0], kt.ap()[0], vv.ap()[0]
        nc.sync.dma_start(kt_sb[0][:, 0:256], ka[:, 0:256])
        nc.scalar.dma_start(qt_sb[0][:, 0:1024], qa[:, 0:1024])
        nc.gpsimd.dma_start(v_sb[0][:, 0:256], va[:, 0:256])
        nc.sync.dma_start(ones_sb[:], ones.ap())
        nc.sync.dma_start(kt_sb[0][:, 256:1024], ka[:, 256:1024])
        nc.gpsimd.dma_start(v_sb[0][:, 256:1024], va[:, 256:1024])
        nc.sync.dma_start(kt_sb[0][:, 1024:2048], ka[:, 1024:2048])
        nc.gpsimd.dma_start(qt_sb[0][:, 1024:2048], qa[:, 1024:2048])
        nc.gpsimd.dma_start(v_sb[0][:, 1024:2048], va[:, 1024:2048])
        for b in range(1, BPC):
            nc.sync.dma_start(kt_sb[b][:], kt.ap()[b])
            nc.gpsimd.dma_start(qt_sb[b][:], qt.ap()[b])
            nc.gpsimd.dma_start(v_sb[b][:], vv.ap()[b])

        # ---- deferred per-half epilogue (rcp + normalize + store) ------
        def make_finish(ps_o, den, osl, out_dma):
            def emit():
                rcp = acc_pool.tile([128, HW_], f32, tag="rcp")
                for j in range(HW_ // 512):
                    jj = slice(j * 512, (j + 1) * 512)
                    nc.vector.reciprocal_approx_fast(out=rcp[:, jj], in_=den[:, jj])
                    nc.vector.scalar_tensor_tensor(
                        osl[:, jj], ps_o[:, jj], 1.0, rcp[:, jj],
                        op0=mybir.AluOpType.mult, op1=mybir.AluOpType.mult,
                    )
                    out_dma(j)
            return emit

        pending_finish = None

        def flush_finish():
            nonlocal pending_finish
            if pending_finish is not None:
                pending_finish()
                pending_finish = None

        # ---- main software-pipelined chunk stream ----------------------
        pending_pv = None

        def flush_pv():
            nonlocal pending_pv
            if pending_pv is not None:
                pending_pv()
                pending_pv = None

        halves = [(b, h) for b in range(BPC) for h in range(NH)]
        st = {}
        for hi, (b, h) in enumerate(halves):
            q0 = h * HW_
            is_final = hi == len(halves) - 1
            ps_o = psO.tile([128, HW_], f32, tag="psO")
            if h == 0:
                st["ot_sb"] = out_pool.tile([128, SQ], f32, tag="ot",
                                            name=f"ot_sb{b}")
            ot_sb = st["ot_sb"]
            acc_a = acc_pool.tile([128, HW_], bf16, tag="acca")
            acc_b = acc_pool.tile([128, HW_], bf16, tag="accb")
            es_prev = None
            raw_mms = []   # (src, start, stop): final-half sums matmuls
            raw_es = None
            ps_b = None

            for k in range(KC):
                ps_s = psS.tile([128, HW_], f32, tag="S")
                for j in range(HW_ // 512):
                    nc.tensor.matmul(
                        ps_s[:, j * 512:(j + 1) * 512],
                        kt_sb[b][:, k * 128:(k + 1) * 128],
                        qt_sb[b][:, q0 + j * 512:q0 + (j + 1) * 512],
                        start=True, stop=True,
                    )
                if k == 10:
                    flush_finish()
                es = es_pool.tile([128, HW_], bf16, tag="es")
                nc.scalar.activation(es[:], ps_s[:], Exp, scale=scale)

                flush_pv()

                def pv(es=es, k=k, ps_o=ps_o, vt=v_sb[b]):
                    for j in range(HW_ // 512):
                        nc.tensor.matmul(
                            ps_o[:, j * 512:(j + 1) * 512],
                            vt[:, k * 128:(k + 1) * 128],
                            es[:, j * 512:(j + 1) * 512],
                            start=(k == 0), stop=(k == KC - 1),
                        )
                pending_pv = pv

                # softmax-denominator partials: bf16 chain adds on DVE
                if k == 0 or k == ACC_SPLIT:
                    pass
                elif k < ACC_SPLIT:
                    nc.vector.tensor_add(
                        acc_a[:], es_prev if k == 1 else acc_a[:], es[:])
                elif k < KC - NRAW:
                    nc.vector.tensor_add(
                        acc_b[:], es_prev if k == ACC_SPLIT + 1 else acc_b[:], es[:])
                else:
                    # raw chunk: summed into the denominator at half end
                    raw_es = es
                    if is_final and k == KC - 1:
                        # final half: partition-reduce via ones-matmuls; emit
                        # the acc sums now (after QK15 so they don't delay
                        # it); the tail chain is then just
                        # exp15 -> 2 matmuls -> rcp -> stt -> store
                        ps_b = psS.tile([128, HW_], f32, tag="S")
                        for srct, start, stop in (
                                (acc_a[:], True, False), (acc_b[:], False, False)):
                            for j in range(HW_ // 512):
                                nc.tensor.matmul(
                                    ps_b[:, j * 512:(j + 1) * 512],
                                    ones_sb[:],
                                    srct[:, j * 512:(j + 1) * 512],
                                    start=start, stop=stop,
                                )
                        raw_mms = [(es[:], False, True)]
                es_prev = es[:]

            dram_half = oo.ap()[b][:, q0:q0 + HW_]
            tile_half = ot_sb[:, q0:q0 + HW_]

            def out_dma(j, dram_half=dram_half, tile_half=tile_half,
                        is_final=is_final):
                jj = slice(j * 512, (j + 1) * 512)
                eng = nc.gpsimd if (is_final and j == 1) else nc.sync
                eng.dma_start(dram_half[:, jj], tile_half[:, jj])

            if is_final:
                # final half: emit es15 sums before PV15 so rcp starts early
                for srct, start, stop in raw_mms:
                    for j in range(HW_ // 512):
                        nc.tensor.matmul(
                            ps_b[:, j * 512:(j + 1) * 512],
                            ones_sb[:],
                            srct[:, j * 512:(j + 1) * 512],
                            start=start, stop=stop,
                        )
                flush_pv()
                make_finish(ps_o, ps_b, tile_half, out_dma)()
            else:
                # denominator: two bf16 adds on DVE, then a GPSIMD
                # partition-all-reduce (no PSUM, no PE work); rcp/normalize
                # deferred into the next half so the DVE queue isn't blocked
                # behind the ~7us reduce
                with tc.high_priority(offset=-10):
                    tsum = acc_pool.tile([128, HW_], bf16, tag="tsum",
                                         name=f"tsum{hi}")
                    nc.vector.tensor_add(tsum[:], acc_a[:], acc_b[:])
                    nc.vector.tensor_add(tsum[:], tsum[:], raw_es[:])
                    den = acc_pool.tile([128, HW_], f32, tag="den",
                                        name=f"den{hi}")
                    nc.gpsimd.partition_all_reduce(
                        den[:], tsum[:], 128, bass_isa.ReduceOp.add)
                pending_finish = make_finish(ps_o, den, tile_half, out_dma)

    nc.compile()
    return nc


_BUILD_CACHE = {}


def _get_nc(scale: float):
    key = round(float(scale), 9)
    if key not in _BUILD_CACHE:
        _BUILD_CACHE[key] = build(float(scale))
    return _BUILD_CACHE[key]


def kernel(x1, x2, x3, x4=None, scale_factor=None, **_ignored):
    import ml_dtypes
    x1 = np.asarray(x1, dtype=np.float32)
    x2 = np.asarray(x2, dtype=np.float32)
    x3 = np.asarray(x3, dtype=np.float32)
    scale = float(np.asarray(scale_factor).reshape(-1)[0])

    # host prep: transpose Q,K to [d, s] fp16; interleave V rows to bf16
    qt = x1.transpose(0, 2, 1).astype(np.float16)               # [B, 128, SQ]
    kt = x2.transpose(0, 2, 1).astype(np.float16)               # [B, 128, SK]
    v = x3.reshape(B, KC, 128, D).transpose(0, 2, 1, 3).reshape(
        B, 128, KC * D).astype(ml_dtypes.bfloat16)              # [B, 128, SK]
    ones = np.ones((128, 128), dtype=ml_dtypes.bfloat16)

    nc = _get_nc(scale)
    in_maps = []
    for c in range(NCORES):
        s = slice(c * BPC, (c + 1) * BPC)
        in_maps.append({
            "qt": np.ascontiguousarray(qt[s]),
            "kt": np.ascontiguousarray(kt[s]),
            "v": np.ascontiguousarray(v[s]),
            "ones": ones,
        })

    trace = bool(int(os.environ.get("KERNEL_TRACE", "0")))
    kwargs = {}
    if trace:
        _install_ntff_hook()
        if bool(int(os.environ.get("KERNEL_TRACE_ALL", "0"))):
            os.environ["BASS_PERFETTO_PROFILE_ALL_CORES"] = "1"
        kwargs = dict(trace=True, trace_kwargs={"title": "attention"})
    res = run_bass_kernel_spmd(nc, in_maps, core_ids=list(range(NCORES)), **kwargs)
    if trace:
        kernel.last_exec_ns = res.exec_time_ns
        kernel.last_trace = res.instructions_and_trace
        kernel.last_mean_exec_ns = res.mean_exec_time_ns

    outT = np.stack([r["o"] for r in res.results])              # [8, BPC, 128, SQ]
    out = outT.reshape(B, 128, SQ).transpose(0, 2, 1)           # [B, SQ, 128]
    return np.ascontiguousarray(out, dtype=np.float32)


kernel.last_exec_ns = None
kernel.last_trace = None
kernel.last_mean_exec_ns = None


# revision 23
# speedup vs baseline: 1.0074x; 1.0074x over previous
"""Trainium2 Bass kernel: batched scaled-dot-product attention.

reference: out[b] = softmax(scale * x1[b] @ x2[b].T) @ x3[b]
shapes: x1,x2,x3 = [16, 2048, 128] fp32.

Sharding: B=16 batches data-parallel over 8 NeuronCores (2 batches/core).

Device algorithm (per batch, per q-half of 1024), software-pipelined over a
single global chunk stream so the PE never parks behind a PV matmul that is
waiting on its exp:
  chunk k:  S^T[k,q] = matmul(lhsT=K^T chunk fp16, rhs=Q^T half fp16)  (FWL)
            es(k)    = exp(scale * S^T)  PSUM -> SBUF bf16   ScalarE
            PV(k-1): outT[dv,q] += matmul(lhsT=V chunk bf16, rhs=es(k-1))
  softmax denominators: bf16 chain adds on DVE (2x perf mode) into two
  accumulators per half + last two chunks raw; ones-matmul reduces over
  partitions and broadcasts; rcp on DVE; normalize + store per 512 cols.

Precision: QK in fp16 (10-bit mantissa; scores |s|<~35 so no overflow),
V/es in bf16. Measured end-to-end rel err ~3e-3 vs the 2e-2 gate.

Host side does layout only: pre-transpose Q,K -> fp16; interleave V rows ->
bf16; transpose the output back. All FLOPs run on device.
"""
import os
import sys
import types
import numpy as np
from contextlib import ExitStack

import concourse.bass as bass  # noqa: F401
import concourse.bass_isa as bass_isa
from concourse import bacc
import concourse.mybir as mybir
import concourse.tile as tile
from concourse.tile_rust import add_dep_helper
import concourse.bass_utils as bass_utils
from concourse.bass_utils import run_bass_kernel_spmd

f32 = mybir.dt.float32
f32r = mybir.dt.float32r
f16 = mybir.dt.float16
bf16 = mybir.dt.bfloat16

B, SQ, SK, D = 16, 2048, 2048, 128
NCORES = 8
BPC = B // NCORES  # batches per core
KC = SK // 128     # k chunks
NH = 2             # q halves
HW_ = SQ // NH     # 1024
NRAW = 1           # last NRAW chunks fed raw to the sums matmul
ACC_SPLIT = 8      # chunks 1..ACC_SPLIT-1 -> acc_a, ACC_SPLIT..KC-NRAW-1 -> acc_b


def _patch_ldw_opt():
    """Enable walrus LDWEIGHTS optimization (background weight-buffer overlap)."""
    if getattr(bass_utils, "_ldw_patched", False):
        return
    # fp16/bf16 weights get FWL (background weight load) from the compiler;
    # the walrus ldw-opt flag is both unnecessary and incompatible with it.
    if not bool(int(os.environ.get("KERNEL_LDWOPT", "0"))):
        return
    orig = bass_utils.run_command

    def patched(argv, **kw):
        if isinstance(argv, list):
            argv = [
                "--enable-ldw-opt=true" if a == "--enable-ldw-opt=false" else a
                for a in argv
            ]
        return orig(argv, **kw)

    bass_utils.run_command = patched
    bass_utils._ldw_patched = True


def _install_ntff_hook():
    """Register the axon NTFF profile hook (used only when tracing)."""
    try:
        from antenv import axon_hooks  # noqa: F401
        return
    except ImportError:
        pass
    try:
        m = types.ModuleType("antenv.axon_hooks")
        m._hook = None
        m.set_axon_ntff_profile_hook = lambda h: setattr(m, "_hook", h)
        m.get_axon_ntff_profile_hook = lambda: m._hook
        sys.modules["antenv.axon_hooks"] = m
        import antenv
        antenv.axon_hooks = m
        from trn_agent_boot.trn_boot import _ntff_profile_via_ctypes
        m._hook = _ntff_profile_via_ctypes("/opt/axon/libaxon_pjrt.so")
    except Exception:
        pass


def build(scale: float):
    _patch_ldw_opt()
    nc = bacc.Bacc("TRN2", target_bir_lowering=False, debug=False)
    qt = nc.dram_tensor("qt", [BPC, 128, SQ], f16, kind="ExternalInput")
    kt = nc.dram_tensor("kt", [BPC, 128, SK], f16, kind="ExternalInput")
    vv = nc.dram_tensor("v", [BPC, 128, SK], bf16, kind="ExternalInput")
    ones = nc.dram_tensor("ones", [128, 128], bf16, kind="ExternalInput")
    oo = nc.dram_tensor("o", [BPC, 128, SQ], f32, kind="ExternalOutput")

    Exp = mybir.ActivationFunctionType.Exp

    with tile.TileContext(nc) as tc, ExitStack() as ctx:
        inp = ctx.enter_context(tc.tile_pool(name="inp", bufs=2))
        es_pool = ctx.enter_context(tc.tile_pool(name="es", bufs=12))
        acc_pool = ctx.enter_context(tc.tile_pool(name="acc", bufs=2))
        out_pool = ctx.enter_context(tc.tile_pool(name="out", bufs=2))
        cpool = ctx.enter_context(tc.tile_pool(name="const", bufs=1))
        psS = ctx.enter_context(tc.tile_pool(name="psS", bufs=2, space="PSUM"))
        psO = ctx.enter_context(tc.tile_pool(name="psO", bufs=2, space="PSUM"))

        ones_sb = cpool.tile([128, 128], bf16, tag="ones")
        tail_prio = int(os.environ.get("KERNEL_TAILPRIO", "40"))

        # ---- input DMA: spread issue across Sync/Vector/GpSimd queues so the
        # first chunk's operands land ~2us in instead of serializing on Sync.
        qt_sb, kt_sb, v_sb = [None] * BPC, [None] * BPC, [None] * BPC
        for b in range(BPC):
            qt_sb[b] = inp.tile([128, SQ], f16, tag="qt", name=f"qt_sb{b}")
            kt_sb[b] = inp.tile([128, SK], f16, tag="kt", name=f"kt_sb{b}")
            v_sb[b] = inp.tile([128, SK], bf16, tag="v", name=f"v_sb{b}")
        qa, ka, va = qt.ap()[0], kt.ap()[0], vv.ap()[0]
        nc.sync.dma_start(kt_sb[0][:, 0:256], ka[:, 0:256])
        nc.scalar.dma_start(qt_sb[0][:, 0:1024], qa[:, 0:1024])
        nc.gpsimd.dma_start(v_sb[0][:, 0:256], va[:, 0:256])
        nc.sync.dma_start(ones_sb[:], ones.ap())
        nc.sync.dma_start(kt_sb[0][:, 256:1024], ka[:, 256:1024])
        nc.gpsimd.dma_start(v_sb[0][:, 256:1024], va[:, 256:1024])
        nc.sync.dma_start(kt_sb[0][:, 1024:2048], ka[:, 1024:2048])
        nc.gpsimd.dma_start(qt_sb[0][:, 1024:2048], qa[:, 1024:2048])
        nc.gpsimd.dma_start(v_sb[0][:, 1024:2048], va[:, 1024:2048])
        for b in range(1, BPC):
            nc.sync.dma_start(kt_sb[b][:], kt.ap()[b])
            nc.gpsimd.dma_start(qt_sb[b][:], qt.ap()[b])
            nc.gpsimd.dma_start(v_sb[b][:], vv.ap()[b])

        # ---- deferred per-half epilogue (rcp + normalize + store) ------
        def make_finish(ps_o, den, osl, out_dma):
            def emit():
                rcp = acc_pool.tile([128, HW_], f32, tag="rcp")
                for j in range(HW_ // 512):
                    jj = slice(j * 512, (j + 1) * 512)
                    nc.vector.reciprocal_approx_fast(out=rcp[:, jj], in_=den[:, jj])
                    nc.vector.scalar_tensor_tensor(
                        osl[:, jj], ps_o[:, jj], 1.0, rcp[:, jj],
                        op0=mybir.AluOpType.mult, op1=mybir.AluOpType.mult,
                    )
                    out_dma(j)
            return emit

        pending_finish = None   # (flush_at_k, emit_fn)
        pending_sums = None

        # ---- main software-pipelined chunk stream ----------------------
        pending_pv = None

        def flush_pv():
            nonlocal pending_pv
            if pending_pv is not None:
                pending_pv()
                pending_pv = None

        halves = [(b, h) for b in range(BPC) for h in range(NH)]
        st = {}
        for hi, (b, h) in enumerate(halves):
            q0 = h * HW_
            is_final = hi == len(halves) - 1
            # last two halves: PE ones-matmul partition reduction (short
            # latency); earlier halves: GPSIMD partition_all_reduce, fully
            # overlapped with the next half
            mm_tail = hi >= len(halves) - 2
            ps_o = psO.tile([128, HW_], f32, tag="psO")
            if h == 0:
                st["ot_sb"] = out_pool.tile([128, SQ], f32, tag="ot",
                                            name=f"ot_sb{b}")
            ot_sb = st["ot_sb"]
            acc_a = acc_pool.tile([128, HW_], bf16, tag="acca")
            acc_b = acc_pool.tile([128, HW_], bf16, tag="accb")
            es_prev = None
            raw_mms = []   # (src, start, stop): final-half sums matmuls
            raw_es = None
            ps_b = None

            for k in range(KC):
                ps_s = psS.tile([128, HW_], f32, tag="S")
                for j in range(HW_ // 512):
                    nc.tensor.matmul(
                        ps_s[:, j * 512:(j + 1) * 512],
                        kt_sb[b][:, k * 128:(k + 1) * 128],
                        qt_sb[b][:, q0 + j * 512:q0 + (j + 1) * 512],
                        start=True, stop=True,
                    )
                if pending_finish is not None and k == pending_finish[0]:
                    pending_finish[1]()
                    pending_finish = None
                es = es_pool.tile([128, HW_], bf16, tag="es")
                nc.scalar.activation(es[:], ps_s[:], Exp, scale=scale)

                flush_pv()
                if k == 0 and pending_sums is not None:
                    pending_sums()
                    pending_sums = None

                def pv(es=es, k=k, ps_o=ps_o, vt=v_sb[b]):
                    for j in range(HW_ // 512):
                        nc.tensor.matmul(
                            ps_o[:, j * 512:(j + 1) * 512],
                            vt[:, k * 128:(k + 1) * 128],
                            es[:, j * 512:(j + 1) * 512],
                            start=(k == 0), stop=(k == KC - 1),
                        )
                pending_pv = pv

                # softmax-denominator partials: bf16 chain adds on DVE
                if k == 0 or k == ACC_SPLIT:
                    pass
                elif k < ACC_SPLIT:
                    nc.vector.tensor_add(
                        acc_a[:], es_prev if k == 1 else acc_a[:], es[:])
                elif k < KC - NRAW:
                    nc.vector.tensor_add(
                        acc_b[:], es_prev if k == ACC_SPLIT + 1 else acc_b[:], es[:])
                else:
                    # raw chunk: summed into the denominator at half end
                    raw_es = es
                    if mm_tail and k == KC - 1:
                        # partition-reduce via ones-matmuls; emit the acc
                        # sums now (after QK15 so they don't delay it); the
                        # remaining chain is exp15 -> 2 matmuls -> rcp -> stt
                        ps_b = psS.tile([128, HW_], f32, tag="S",
                                        name=f"ps_b{hi}")
                        for srct, start, stop in (
                                (acc_a[:], True, False), (acc_b[:], False, False)):
                            for j in range(HW_ // 512):
                                nc.tensor.matmul(
                                    ps_b[:, j * 512:(j + 1) * 512],
                                    ones_sb[:],
                                    srct[:, j * 512:(j + 1) * 512],
                                    start=start, stop=stop,
                                )
                        raw_mms = [(es[:], False, True)]
                es_prev = es[:]

            dram_half = oo.ap()[b][:, q0:q0 + HW_]
            tile_half = ot_sb[:, q0:q0 + HW_]

            def out_dma(j, dram_half=dram_half, tile_half=tile_half,
                        is_final=is_final):
                jj = slice(j * 512, (j + 1) * 512)
                eng = nc.gpsimd if (is_final and j == 1) else nc.sync
                eng.dma_start(dram_half[:, jj], tile_half[:, jj])

            if is_final:
                # final half: emit es15 sums before PV15 so rcp starts early
                for srct, start, stop in raw_mms:
                    for j in range(HW_ // 512):
                        nc.tensor.matmul(
                            ps_b[:, j * 512:(j + 1) * 512],
                            ones_sb[:],
                            srct[:, j * 512:(j + 1) * 512],
                            start=start, stop=stop,
                        )
                flush_pv()
                make_finish(ps_o, ps_b, tile_half, out_dma)()
            elif mm_tail:
                # es15 sums + finish ride into the next half's stream
                def sums(raw_mms=raw_mms, ps_b=ps_b):
                    for srct, start, stop in raw_mms:
                        for j in range(HW_ // 512):
                            nc.tensor.matmul(
                                ps_b[:, j * 512:(j + 1) * 512],
                                ones_sb[:],
                                srct[:, j * 512:(j + 1) * 512],
                                start=start, stop=stop,
                            )
                pending_sums = sums
                pending_finish = (1, make_finish(ps_o, ps_b, tile_half, out_dma))
            else:
                # denominator: two bf16 adds on DVE, then a GPSIMD
                # partition-all-reduce (no PSUM, no PE work); rcp/normalize
                # deferred into the next half so the DVE queue isn't blocked
                # behind the ~7us reduce
                with tc.high_priority(offset=-10):
                    tsum = acc_pool.tile([128, HW_], bf16, tag="tsum",
                                         name=f"tsum{hi}")
                    nc.vector.tensor_add(tsum[:], acc_a[:], acc_b[:])
                    nc.vector.tensor_add(tsum[:], tsum[:], raw_es[:])
                    den = acc_pool.tile([128, HW_], f32, tag="den",
                                        name=f"den{hi}")
                    nc.gpsimd.partition_all_reduce(
                        den[:], tsum[:], 128, bass_isa.ReduceOp.add)
                pending_finish = (10, make_finish(ps_o, den, tile_half, out_dma))

    nc.compile()
    return nc


_BUILD_CACHE = {}


def _get_nc(scale: float):
    key = round(float(scale), 9)
    if key not in _BUILD_CACHE:
        _BUILD_CACHE[key] = build(float(scale))
    return _BUILD_CACHE[key]


def kernel(x1, x2, x3, x4=None, scale_factor=None, **_ignored):
    import ml_dtypes
    x1 = np.asarray(x1, dtype=np.float32)
    x2 = np.asarray(x2, dtype=np.float32)
    x3 = np.asarray(x3, dtype=np.float32)
    scale = float(np.asarray(scale_factor).reshape(-1)[0])

    # host prep: transpose Q,K to [d, s] fp16; interleave V rows to bf16
    qt = x1.transpose(0, 2, 1).astype(np.float16)               # [B, 128, SQ]
    kt = x2.transpose(0, 2, 1).astype(np.float16)               # [B, 128, SK]
    v = x3.reshape(B, KC, 128, D).transpose(0, 2, 1, 3).reshape(
        B, 128, KC * D).astype(ml_dtypes.bfloat16)              # [B, 128, SK]
    ones = np.ones((128, 128), dtype=ml_dtypes.bfloat16)

    nc = _get_nc(scale)
    in_maps = []
    for c in range(NCORES):
        s = slice(c * BPC, (c + 1) * BPC)
        in_maps.append({
            "qt": np.ascontiguousarray(qt[s]),
            "kt": np.ascontiguousarray(kt[s]),
            "v": np.ascontiguousarray(v[s]),
            "ones": ones,
        })

    trace = bool(int(os.environ.get("KERNEL_TRACE", "0")))
    kwargs = {}
    if trace:
        _install_ntff_hook()
        if bool(int(os.environ.get("KERNEL_TRACE_ALL", "0"))):
            os.environ["BASS_PERFETTO_PROFILE_ALL_CORES"] = "1"
        kwargs = dict(trace=True, trace_kwargs={"title": "attention"})
    res = run_bass_kernel_spmd(nc, in_maps, core_ids=list(range(NCORES)), **kwargs)
    if trace:
        kernel.last_exec_ns = res.exec_time_ns
        kernel.last_trace = res.instructions_and_trace
        kernel.last_mean_exec_ns = res.mean_exec_time_ns

    outT = np.stack([r["o"] for r in res.results])              # [8, BPC, 128, SQ]
    out = outT.reshape(B, 128, SQ).transpose(0, 2, 1)           # [B, SQ, 128]
    return np.ascontiguousarray(out, dtype=np.float32)


kernel.last_exec_ns = None
kernel.last_trace = None
kernel.last_mean_exec_ns = None
